# revision 2
# baseline (speedup 1.0000x reference)
"""Distributed multi-head attention kernel for Trainium2 (8 NeuronCores).

Problem: B=2, S=2048, D=1024, H=16 heads, DH=64.
  qkv = x @ w_qkv + b_qkv ; per-head softmax(q k^T / 8) v ; out proj.

Sharding (batch x head-group tensor parallel):
  core c = g*4 + j handles batch g and heads 4j..4j+3.  Each core
  computes q/k (transposed layout) + v projections for its heads,
  transposed-score attention, then one 8-core AllToAll per head pair
  exchanges attention outputs so each core finishes the output
  projection for s-block c (256 rows of EACH batch) with the full
  1024-dim contraction.  The host only concatenates disjoint slices.

Stage-A schedule (vs. earlier baseline):
  - Coarse multi-dim DMA descriptors: one issue per weight tensor and
    per x d-tile (the DGE issue path costs ~0.6us per descriptor on
    the issuing engine, so fewer/bigger transfers start the ramp much
    earlier).
  - Phase A consumes each x d-tile as it lands with all four live
    PSUM accumulators (q/k pair 0 x two s-halves), so the PE tracks
    the DMA ramp instead of waiting for full passes.
  - A memset-seeded heater warms the PE p-state before the first real
    matmul and again across the final AllToAll wait.
  - Output staged and DMA'd as bf16 (host upcasts); a2a_in / aout
    bounce transfers are single rearranged descriptors instead of
    per-slice DMAs.

Layout trick: scores are computed transposed (scoresT[k,q] = kT.T@qT
with both operands in [dh, s] layout straight out of the qk
projection), so the exp output feeds attn@v as the *moving* operand,
and a ones column appended to v yields the softmax row-sums as a 65th
output row of the same matmul.  Matmul operands are bf16 (fp32 PSUM
accumulation).
"""

import numpy as np

import concourse.bacc as bacc
import concourse.mybir as mybir
import concourse.tile as tile
from concourse import bass_utils

F32 = mybir.dt.float32
BF16 = mybir.dt.bfloat16
EXP = mybir.ActivationFunctionType.Exp
MULT = mybir.AluOpType.mult

B, S, D, H = 2, 2048, 1024, 16
DH = D // H            # 64
NCORE = 8
GRP = 4                # cores per batch group
HL = H // GRP          # 4 local heads per core
DTILES = D // 128      # 8 contraction chunks
STILES = S // 128      # 16
SBW = S // NCORE       # 256: AllToAll s-block width
VW = DH + 1            # 65: v columns + ones column
VP = 128               # padded v block: [v(64) | ones(1) | zeros(63)]

_CACHE = {}


def _build():
    nc = bacc.Bacc("TRN2", target_bir_lowering=False, debug=False,
                   num_devices=NCORE)

    xT_d = nc.dram_tensor("xT", [D, S], BF16, kind="ExternalInput")
    wqk_d = nc.dram_tensor("wqk", [D, 2 * HL * DH], BF16, kind="ExternalInput")
    wv_d = nc.dram_tensor("wv", [D, HL * DH], BF16, kind="ExternalInput")
    bqk_d = nc.dram_tensor("bqk", [2 * HL * DH], F32, kind="ExternalInput")
    bv_d = nc.dram_tensor("bv", [HL * DH], F32, kind="ExternalInput")
    wout_d = nc.dram_tensor("wout", [D, D], BF16, kind="ExternalInput")
    bout_d = nc.dram_tensor("bout", [D], F32, kind="ExternalInput")
    ident_d = nc.dram_tensor("ident", [128, 128], BF16, kind="ExternalInput")
    out_d = nc.dram_tensor("out", [2 * SBW, D], BF16, kind="ExternalOutput")

    groups = [list(range(NCORE))]

    with tile.TileContext(nc) as tc:
        with (
            tc.tile_pool(name="persist", bufs=1) as pers,
            tc.tile_pool(name="big", bufs=DTILES) as big,
            tc.tile_pool(name="wsmall", bufs=1) as wsmall,
            tc.tile_pool(name="ppool", bufs=8) as ppool,
            tc.tile_pool(name="npool", bufs=2) as npool,
            tc.tile_pool(name="fin", bufs=2) as fin,
            tc.tile_pool(name="dram", bufs=1, space="DRAM") as dram,
        ):
            # ---- persistent SBUF tensors ----
            kT = pers.tile([128, 2 * S], BF16, tag="kT")
            qp = pers.tile([128, 4 * S], BF16, tag="qp")
            vext = pers.tile([128, STILES * HL * VP], BF16, tag="vext")
            aout = pers.tile([128, 16 * SBW], BF16, tag="aout")
            outacc = pers.tile([128, 4 * D], BF16, tag="outacc")
            bqk_sb = pers.tile([128, 4], F32, tag="bqk_sb")
            bv_sb = pers.tile([128, HL * DH], F32, tag="bv_sb")
            bv_row = ppool.tile([1, HL * DH], F32, tag="P", name="bv_row")
            bout_bf = pers.tile([128, D], BF16, tag="bout_bf")
            bout_row = ppool.tile([1, D], F32, tag="P", name="bout_row")
            ident = pers.tile([128, 128], BF16, tag="ident")
            e0m = pers.tile([128, 128], BF16, tag="e0m")

            wqk_sb = wsmall.tile([128, DTILES * 512], BF16, tag="wqk_sb")
            wv_sb = wsmall.tile([128, DTILES * 256], BF16, tag="wv_sb")
            wout_sb = wsmall.tile([128, DTILES * D], BF16, tag="wout_sb")

            # ---- PE heater seed (memset, no DMA dependency) ----
            heat_b = wsmall.tile([128, 512], BF16, tag="heat_b")
            nc.vector.memset(heat_b[:], 0.25)

            # ---- input DMAs: coarse descriptors, spread over the two
            # issuing engines (sync first: x/wqk are phase-A critical).
            engs = [nc.sync, nc.gpsimd]
            xt_tiles = [big.tile([128, S], BF16, tag="big", name=f"xt{dt}")
                        for dt in range(DTILES)]
            # interleave wqk chunk + x tile per dt, alternating queues so
            # arrival order is ~ 0,4,1,5,2,6,3,7
            for dt in range(DTILES):
                q = engs[0] if dt < 4 else engs[1]
                q.dma_start(
                    wqk_sb[:, dt * 512:(dt + 1) * 512],
                    wqk_d[dt * 128:(dt + 1) * 128, :])
                q.dma_start(xt_tiles[dt][:], xT_d[dt * 128:(dt + 1) * 128, :])
            DT_ORDER = [0, 4, 1, 5, 2, 6, 3, 7]
            # wv: one descriptor
            engs[1].dma_start(
                wv_sb[:].rearrange("p (d c) -> p d c", c=256),
                wv_d[:].rearrange("(d p) c -> p d c", p=128))
            # wout: one descriptor (lands during attention, used at tail)
            engs[0].dma_start(
                wout_sb[:].rearrange("p (d c) -> p d c", c=D),
                wout_d[:].rearrange("(d p) c -> p d c", p=128))
            # biases
            nc.sync.dma_start(bqk_sb[:],
                              bqk_d[:].rearrange("(e p) -> p e", p=128))
            nc.gpsimd.dma_start(bv_row[:], bv_d[:].unsqueeze(0))
            nc.gpsimd.partition_broadcast(bv_sb[:], bv_row[:1, :])
            nc.gpsimd.dma_start(bout_row[:], bout_d[:].unsqueeze(0))
            nc.vector.memset(bout_bf[:], 0.0)
            nc.vector.tensor_copy(bout_bf[0:1, :], bout_row[:1, :])
            nc.gpsimd.dma_start(ident[:], ident_d[:])
            nc.vector.memset(e0m[:], 0.0)
            nc.vector.memset(e0m[0:1, :], 1.0)
            # vext: zero cols 65-127, ones col 64 of each [*,128] block
            nc.vector.memset(
                vext[:].rearrange("p (b w) -> p b w", w=VP)[:, :, DH + 1:VP],
                0.0)
            nc.vector.memset(
                vext[:].rearrange("p (b w) -> p b w", w=VP)[:, :, DH:DH + 1],
                1.0)
            # qp zero halves
            for pr in range(2):
                nc.vector.memset(qp[64:128, (2 * pr) * S:(2 * pr + 1) * S], 0.0)
                nc.vector.memset(qp[0:64, (2 * pr + 1) * S:(2 * pr + 2) * S], 0.0)

            # ---- pre-warm heater: runs while the first DMAs land ----
            NHEAT = 40
            if NHEAT:
                with tc.tile_pool(name="psH", bufs=1, space="PSUM") as psH:
                    ph = psH.tile([128, 512], F32, tag="psH")
                    for i in range(NHEAT):
                        nc.tensor.matmul(ph[:, 0:256], heat_b[:, 0:128],
                                         heat_b[:, 0:256],
                                         start=True, stop=True)

            # ---- projection helpers ----
            def qk_store(et, sh, acc):
                dsts = slice(sh * 1024, (sh + 1) * 1024)
                if et >= 2:      # k pair et-2: full 128 rows
                    pr = et - 2
                    nc.vector.tensor_scalar_add(
                        kT[:, pr * S:(pr + 1) * S][:, dsts],
                        acc[:], bqk_sb[:, et:et + 1])
                else:            # q pair et: split into padded tiles
                    pr = et
                    nc.vector.tensor_scalar_add(
                        qp[0:64, (2 * pr) * S:(2 * pr + 1) * S][:, dsts],
                        acc[0:64, :], bqk_sb[0:64, et:et + 1])
                    nc.vector.tensor_scalar_add(
                        qp[64:128, (2 * pr + 1) * S:(2 * pr + 2) * S][:, dsts],
                        acc[64:128, :], bqk_sb[64:128, et:et + 1])

            def qk_chunk(pool, tag, et, sh, c):
                # one 512-wide c-chunk of a q/k projection
                acc = pool.tile([128, 512], F32, tag=tag,
                                name=f"qk{et}{sh}{c}")
                sl = slice(sh * 1024 + c * 512, sh * 1024 + (c + 1) * 512)
                for dt in range(DTILES):
                    nc.tensor.matmul(
                        acc[:],
                        wqk_sb[:, dt * 512 + et * 128:
                               dt * 512 + (et + 1) * 128],
                        xt_tiles[dt][:, sl],
                        start=(dt == 0), stop=(dt == DTILES - 1))
                if et >= 2:
                    pr = et - 2
                    nc.vector.tensor_scalar_add(
                        kT[:, pr * S:(pr + 1) * S][:, sl],
                        acc[:], bqk_sb[:, et:et + 1])
                else:
                    pr = et
                    nc.vector.tensor_scalar_add(
                        qp[0:64, (2 * pr) * S:(2 * pr + 1) * S][:, sl],
                        acc[0:64, :], bqk_sb[0:64, et:et + 1])
                    nc.vector.tensor_scalar_add(
                        qp[64:128, (2 * pr + 1) * S:(2 * pr + 2) * S][:, sl],
                        acc[64:128, :], bqk_sb[64:128, et:et + 1])

            def v_acc(pool, tag, st):
                acc = pool.tile([128, HL * DH], F32, tag=tag, name=f"v{st}")
                for dt in range(DTILES):
                    nc.tensor.matmul(
                        acc[:],
                        xt_tiles[dt][:, st * 128:(st + 1) * 128],
                        wv_sb[:, dt * 256:(dt + 1) * 256],
                        start=(dt == 0), stop=(dt == DTILES - 1))
                base = st * HL * VP
                vv = vext[:, base:base + HL * VP].rearrange(
                    "p (h w) -> p h w", h=HL)
                nc.vector.tensor_add(
                    vv[:, :, 0:DH],
                    acc[:].rearrange("p (h w) -> p h w", h=HL),
                    bv_sb[:].rearrange("p (h w) -> p h w", h=HL))

            # ---- phase A: q/k pair 0 with 4 live accumulators, consuming
            # each x d-tile (both s-halves) as its DMA lands. ----
            with tc.tile_pool(name="psA0", bufs=4, space="PSUM") as psA0:
                accs = {(et, sh): psA0.tile([128, 1024], F32, tag="psA0",
                                            name=f"qa{et}{sh}")
                        for et in (0, 2) for sh in range(2)}
                for i, dt in enumerate(DT_ORDER):
                    for et in (0, 2):
                        for sh in range(2):
                            for c in range(2):
                                sl = slice(sh * 1024 + c * 512,
                                           sh * 1024 + (c + 1) * 512)
                                nc.tensor.matmul(
                                    accs[(et, sh)][:, c * 512:(c + 1) * 512],
                                    wqk_sb[:, dt * 512 + et * 128:
                                           dt * 512 + (et + 1) * 128],
                                    xt_tiles[dt][:, sl],
                                    start=(i == 0), stop=(i == DTILES - 1))
                for et in (0, 2):
                    for sh in range(2):
                        qk_store(et, sh, accs[(et, sh)])

            # v tiles 0..7 right after phase A (psA banks freed)
            with tc.tile_pool(name="psV", bufs=4, space="PSUM") as psV:
                for st in range(8):
                    v_acc(psV, "psV", st)

            wout_tiles = [wout_sb[:, ec * D:(ec + 1) * D]
                          for ec in range(DTILES)]

            # ---- attention + AllToAll (8-core; s-blocks of 256) ----
            a2a_in = [dram.tile([NCORE, 128, SBW], BF16, tag=f"a2a_in{p}",
                                name=f"a2a_in{p}") for p in range(2)]
            a2a_out = [dram.tile([NCORE, 128, SBW], BF16, tag=f"a2a_out{p}",
                                 name=f"a2a_out{p}") for p in range(2)]

            with (
                tc.tile_pool(name="psS", bufs=2, space="PSUM") as psS,
                tc.tile_pool(name="psO", bufs=2, space="PSUM") as psO,
            ):
                def outproj_half(p):
                    for gb in range(2):
                        for st in range(SBW // 128):
                            acc = psO.tile([128, D], F32, tag="psO",
                                           name=f"op{gb}{st}")
                            oa = outacc[:, (gb * 2 + st) * D:
                                        (gb * 2 + st + 1) * D]
                            for c in range(2):
                                nc.tensor.matmul(
                                    acc[:, c * 512:(c + 1) * 512],
                                    e0m[:] if p == 0 else ident[:],
                                    (bout_bf if p == 0 else oa)
                                    [:, c * 512:(c + 1) * 512],
                                    start=True, stop=False)
                            for jr in range(GRP):
                                jj = gb * GRP + jr
                                col = (p * NCORE + jj) * SBW + st * 128
                                for c in range(2):
                                    nc.tensor.matmul(
                                        acc[:, c * 512:(c + 1) * 512],
                                        aout[:, col:col + 128],
                                        wout_tiles[p * GRP + jr][:, c * 512:(c + 1) * 512],
                                        start=False, stop=(jr == GRP - 1))
                            if p == 0:
                                nc.scalar.copy(oa, acc[:])
                            else:
                                res = fin.tile([128, D], BF16, tag="res")
                                nc.scalar.copy(res[:], acc[:])
                                row = gb * SBW + st * 128
                                (nc.sync if gb == 0 else nc.gpsimd).dma_start(
                                    out_d[row:row + 128, :], res[:])

                # in-loop PE tasks for the pr=0 iterations
                ptasks = {k: ("v", 8 + k) for k in range(8)}
                qkc = [(et, sh, c) for et in (3, 1) for sh in range(2)
                       for c in range(2)]
                for slot, t in zip((9, 11, 13, 15, 17, 19, 21, 23), qkc):
                    ptasks[slot] = ("qk", t)

                for pr in range(2):      # head pair: lh = 2*pr, 2*pr+1
                    for qh in range(2):
                        po = [psO.tile([128, 1024], F32, tag="psO",
                                       name=f"po{h}") for h in range(2)]
                        prev_p = None
                        for kt in range(STILES):
                            ps2 = [psS.tile([128, 1024], F32, tag="psS",
                                            name=f"ps{h}") for h in range(2)]
                            for h in range(2):
                                for c in range(2):
                                    nc.tensor.matmul(
                                        ps2[h][:, c * 512:(c + 1) * 512],
                                        kT[:, pr * S + kt * 128:
                                           pr * S + (kt + 1) * 128],
                                        qp[:, (2 * pr + h) * S + qh * 1024 +
                                           c * 512:
                                           (2 * pr + h) * S + qh * 1024 +
                                           (c + 1) * 512],
                                        start=True, stop=True)
                            if prev_p is not None:
                                for h in range(2):
                                    vb = ((kt - 1) * HL + 2 * pr + h) * VP
                                    for c in range(2):
                                        nc.tensor.matmul(
                                            po[h][:, c * 512:(c + 1) * 512],
                                            vext[:, vb:vb + VP],
                                            prev_p[h][:, c * 512:(c + 1) * 512],
                                            start=(kt - 1 == 0), stop=False)
                            pexp = [ppool.tile([128, 1024], BF16, tag="P",
                                               name=f"pexp{h}") for h in range(2)]
                            for h in range(2):
                                nc.scalar.activation(pexp[h][:], ps2[h][:],
                                                     EXP, scale=0.125)
                            prev_p = pexp
                            if pr == 0:
                                task = ptasks.get(qh * STILES + kt)
                                if task is not None:
                                    if task[0] == "v":
                                        v_acc(psS, "psS", task[1])
                                    else:
                                        et, sh, c = task[1]
                                        qk_chunk(psS, "psS", et, sh, c)
                        for h in range(2):
                            vb = ((STILES - 1) * HL + 2 * pr + h) * VP
                            for c in range(2):
                                nc.tensor.matmul(
                                    po[h][:, c * 512:(c + 1) * 512],
                                    vext[:, vb:vb + VP],
                                    prev_p[h][:, c * 512:(c + 1) * 512],
                                    start=False, stop=True)
                        last_q = (pr == 1 and qh == 1)
                        if last_q:
                            outproj_half(0)

                        for h in range(2):
                            rs_row = npool.tile([1, 1024], F32, tag="rs_row",
                                                name="rs_row")
                            rs_rec = npool.tile([1, 1024], F32, tag="rs_rec",
                                                name="rs_rec")
                            rs_b = npool.tile([64, 1024], F32, tag="rs_b",
                                              name="rs_b")
                            attn = npool.tile([64, 1024], BF16, tag="attn",
                                              name="attn")
                            nc.vector.tensor_copy(rs_row[:], po[h][DH:VW, :])
                            nc.vector.reciprocal_approx_fast(rs_rec[:],
                                                             rs_row[:1, :])
                            nc.gpsimd.partition_broadcast(rs_b[:], rs_rec[:1, :])
                            if last_q:
                                nc.vector.tensor_tensor(attn[:], po[h][0:DH, :],
                                                        rs_b[:], MULT)
                            else:
                                stg = npool.tile([DH, 1024], F32, tag="stg",
                                                 name="stg")
                                nc.vector.tensor_copy(stg[:], po[h][0:DH, :])
                                nc.vector.tensor_tensor(attn[:], stg[:, :],
                                                        rs_b[:], MULT)
                            # one descriptor: 4 dest slices of this qh-half
                            rr = h * 64
                            eng = nc.gpsimd if (last_q and h == 1) else nc.sync
                            eng.dma_start(
                                a2a_in[pr][qh * 4:(qh + 1) * 4, rr:rr + 64, :]
                                .rearrange("d p c -> p d c"),
                                attn[:].rearrange("p (d c) -> p d c", c=SBW))
                        if last_q:
                            # keep the PE p-state warm across the final
                            # AllToAll wait
                            for i in range(96):
                                hacc = psS.tile([128, 512], F32, tag="psS",
                                                name="hgap") if i % 48 == 0 \
                                    else hacc
                                nc.tensor.matmul(hacc[:, 0:256],
                                                 heat_b[:, 0:128],
                                                 heat_b[:, 0:256],
                                                 start=True, stop=True)
                    nc.gpsimd.collective_compute(
                        "AllToAll", mybir.AluOpType.bypass,
                        replica_groups=groups,
                        ins=[a2a_in[pr][:].opt()],
                        outs=[a2a_out[pr][:].opt()])
                    # one descriptor for the whole pair's aout block
                    nc.sync.dma_start(
                        aout[:, pr * NCORE * SBW:(pr + 1) * NCORE * SBW]
                        .rearrange("p (d c) -> p d c", c=SBW),
                        a2a_out[pr][:].rearrange("d p c -> p d c"))

                outproj_half(1)

    nc.compile()
    return nc


def _shard(inputs):
    import ml_dtypes
    bf = ml_dtypes.bfloat16
    x = np.asarray(inputs["x"], np.float32)
    w_qkv = np.asarray(inputs["w_qkv"], np.float32)
    b_qkv = np.asarray(inputs["b_qkv"], np.float32)
    w_out = np.asarray(inputs["w_out"], np.float32)
    b_out = np.asarray(inputs["b_out"], np.float32)

    # wout rows permuted to match AllToAll output row order:
    # for pair p, peer rank-in-group jr, t in (0,1): head 4*jr + 2*p + t
    rows = []
    for p in (0, 1):
        for jr in range(GRP):
            for t in (0, 1):
                h = 4 * jr + 2 * p + t
                rows.append(w_out[h * DH:(h + 1) * DH, :])
    wout_perm = np.ascontiguousarray(np.concatenate(rows, 0))

    in_maps = []
    for c in range(NCORE):
        g, j = c // GRP, c % GRP
        cs = slice(j * HL * DH, (j + 1) * HL * DH)
        wqk = np.concatenate([w_qkv[:, :D][:, cs], w_qkv[:, D:2 * D][:, cs]], 1)
        bqk = np.concatenate([b_qkv[:D][cs], b_qkv[D:2 * D][cs]])
        in_maps.append({
            "xT": np.ascontiguousarray(x[g].T).astype(bf),
            "wqk": np.ascontiguousarray(wqk).astype(bf),
            "wv": np.ascontiguousarray(w_qkv[:, 2 * D:][:, cs]).astype(bf),
            "bqk": np.ascontiguousarray(bqk),
            "bv": np.ascontiguousarray(b_qkv[2 * D:][cs]),
            "wout": wout_perm.astype(bf),
            "bout": b_out,
            "ident": np.eye(128, dtype=np.float32).astype(bf),
        })
    return in_maps


def _install_ntff_hook():
    """The agent image's antenv lacks axon_hooks; shim it and register the
    ctypes NTFF profiler from trn_agent_boot so trace=True works."""
    import sys
    import types

    if "antenv.axon_hooks" in sys.modules:
        return
    import antenv

    mod = types.ModuleType("antenv.axon_hooks")
    mod._hook = None
    mod.set_axon_ntff_profile_hook = lambda h: setattr(mod, "_hook", h)
    mod.get_axon_ntff_profile_hook = lambda: mod._hook
    sys.modules["antenv.axon_hooks"] = mod
    antenv.axon_hooks = mod
    try:
        from trn_agent_boot.trn_boot import _ntff_profile_via_ctypes
        mod._hook = _ntff_profile_via_ctypes("/opt/axon/libaxon_pjrt.so")
    except Exception as e:  # degrade like upstream: no trace, run still works
        print(f"ntff hook install failed: {e}")


def _run(inputs, trace=False):
    if trace:
        _install_ntff_hook()
    if "nc" not in _CACHE:
        _CACHE["nc"] = _build()
    nc = _CACHE["nc"]
    in_maps = _shard(inputs)
    r = bass_utils.run_bass_kernel_spmd(
        nc, in_maps, core_ids=list(range(NCORE)), trace=trace)
    out = np.empty((B, S, D), np.float32)
    for c in range(NCORE):
        for g in range(B):
            out[g, c * SBW:(c + 1) * SBW, :] = \
                r.results[c]["out"][g * SBW:(g + 1) * SBW].astype(np.float32)
    return out, r


def kernel(**inputs) -> np.ndarray:
    out, _ = _run(inputs, trace=False)
    return out
